# revision 20
# baseline (speedup 1.0000x reference)
"""LIF neuron (leaky integrate-and-fire) Bass kernel for Trainium2.

Reference semantics (per element, recurrence over time axis T=32):
    mem_t   = tau * mem_{t-1} + x_t
    spike_t = 1.0 if mem_t > vth else 0.0
    mem_t   = mem_t * (1 - spike_t)        # hard reset
Input  x: [16, 32, 65536] f32  ->  Output spikes: [16, 32, 65536] f32.
Sharding: data parallel over batch, 8 cores x 2 batch rows each.

Design (v3, measured-engine-balanced):
  The recurrence needs two 2-tensor DVE ops per step (acc, mem) --
  ~77us/pass on DVE at 1 elem/lane/cycle; that's the compute floor and
  the binding engine.  Spike extraction rides ACT: ONE Sign op per
  step writing uint8 directly -- the f32->u8 conversion saturates the
  sign's -1 to 0, so u8(sign(acc-vth)) IS the {0,1} spike exactly
  (~34us ACT).  Stores shrink 4x (16 MiB -> 4 MiB/core) and ride the
  scalar HWDGE ring, overlapping the sync-ring load stream.
  Pool/PE idle: Pool's Q7 software TT is ~4x slower than DVE and
  walrus rejects STT/comparisons on it; PE fp32 matmul is ~1/4 rate.

  Membrane chain is split into two independent batch-row halves
  [128, 512] so the serial acc->mem->acc dependency pipelines on DVE
  (chain ~46us < 77us throughput).

Per step, on a [128, 1024] step-tile (2 batch x 512 d per partition):
  DVE  STT x2: acc_h = (mem_h * tau) + x_h          (per half, f32)
  DVE  STT x2: mem_h' = (acc_h <= vth) * acc_h      (per half, f32)
  ACT  Sign:   spk_u8 = u8(sign(acc - vth))         (uint8 out, exact)
Ring schedule per pass: all 16x1MiB loads issue first on the sync
HWDGE ring; per-group 512KiB uint8 spike stores ride the scalar ring
concurrently.  Full-x SBUF residency (bufs=NG).
"""

import os
import sys

sys.path.insert(0, "/opt/trn_rl_repo")

import numpy as np

from concourse import bacc, mybir, tile
from concourse.bass_utils import run_bass_kernel_spmd

TAU = 0.2
VTH = 0.5

B, T, D = 16, 32, 65536
N_CORES = 8
B_SH = B // N_CORES          # 2 batch rows per core
P = 128                      # SBUF partitions
FB = D // P                  # 512 d-elems per partition per batch row
F = B_SH * FB                # 1024 free elems per step-tile

GS = int(os.environ.get("LIF_GS", "4"))   # timesteps per DMA group
ACC_BUFS = int(os.environ.get("LIF_ACCBUFS", "4"))
MEM_BUFS = int(os.environ.get("LIF_MEMBUFS", "4"))
SPK_BUFS = int(os.environ.get("LIF_SPKBUFS", "3"))
NG = T // GS                 # groups per pass
JG = GS * F                  # per-group free elems (4096)

SPK_DT = os.environ.get("LIF_SPKDT", "u8")   # u8 | f8 | bf16
SPLIT0 = int(os.environ.get("LIF_SPLIT0", "0"))  # per-step DMAs for group 0
ST_RING = os.environ.get("LIF_STRING", "scalar")  # sync | scalar | pool
ACC_FULL = int(os.environ.get("LIF_ACCFULL", "0"))  # acc as one [P,F] op
MEM_FULL = int(os.environ.get("LIF_MEMFULL", "0"))  # mem as one [P,F] op

_progs = {}


def _spk_dt():
    return {"u8": mybir.dt.uint8, "f8": mybir.dt.float8e4,
            "bf16": mybir.dt.bfloat16}[SPK_DT]


def _build_program(hw_loop=None, mode="full"):
    f32 = mybir.dt.float32
    nc = bacc.Bacc(
        "TRN2",
        target_bir_lowering=False,
        debug=False,
        enable_asserts=False,
        num_devices=N_CORES,
    )
    x = nc.dram_tensor("x", [B_SH, T, D], f32, kind="ExternalInput").ap()
    xr = x.rearrange("b (g tl) (p f) -> g p tl b f", tl=GS, p=P)
    out = nc.dram_tensor("out", [NG, P, JG], _spk_dt(),
                         kind="ExternalOutput").ap()

    with tile.TileContext(nc) as tc:
        with (
            # full-x residency: one buffer per group, reused across passes
            tc.tile_pool(name="xt", bufs=NG) as xp,
            tc.tile_pool(name="acc", bufs=ACC_BUFS) as ap_,
            tc.tile_pool(name="spk", bufs=SPK_BUFS) as kp,
            tc.tile_pool(name="m0", bufs=MEM_BUFS) as mp0,
            tc.tile_pool(name="m1", bufs=MEM_BUFS) as mp1,
            tc.tile_pool(name="const", bufs=1) as cp,
        ):
            nvth = cp.tile([P, 1], f32)
            nc.gpsimd.memset(nvth[:], -VTH)

            def body():
                one_pass(nc, xr, out, xp, ap_, kp, (mp0, mp1),
                         nvth, mode)

            if hw_loop is None:
                body()
            else:
                # benchmarking only: repeat the full pass in a HW loop so
                # per-pass device time can be fit from wall-clock deltas
                with tc.For_i(0, hw_loop, 1):
                    body()
    nc.compile()
    return nc


def one_pass(nc, xr, out, xp, ap_, kp, mps, nvth, mode):
    f32 = mybir.dt.float32
    mult = mybir.AluOpType.mult
    add = mybir.AluOpType.add
    is_le = mybir.AluOpType.is_le
    Sign = mybir.ActivationFunctionType.Sign

    # Phase 1: issue every load on the sync ring (back-to-back, no WAR).
    # Group 0 loads are split per-step so the first acc can start after
    # a 256 KiB DMA instead of a 1 MiB one (ramp cut).
    xts = []
    for g in range(NG):
        xt = xp.tile([P, JG], f32)
        if mode == "compute":
            nc.gpsimd.memset(xt[:], 0.125)
        else:
            xt_v = xt[:].rearrange("p (tl b f) -> p tl b f", tl=GS, b=B_SH)
            if g == 0 and SPLIT0:
                for tl in range(GS):
                    for b in range(B_SH):
                        nc.sync.dma_start(out=xt_v[:, tl, b],
                                          in_=xr[g][:, tl, b])
            else:
                for b in range(B_SH):
                    nc.sync.dma_start(out=xt_v[:, :, b], in_=xr[g][:, :, b])
        xts.append(xt)
    if mode == "load":
        return

    # Phase 2: recurrence; uint8 spike stores drain after loads by FIFO
    mem = [None, None]           # per-half membrane tiles [P, FB]
    for g in range(NG):
        xt = xts[g]
        spk = kp.tile([P, JG], _spk_dt())
        for tl in range(GS):
            t = g * GS + tl
            xs = xt[:, tl * F:(tl + 1) * F]
            if t == 0:
                acc = xs         # mem_{-1} = 0 -> acc = x_0
            else:
                acc = ap_.tile([P, F], f32)
                if ACC_FULL:
                    # one full-tile op; mem is a full tile w/ half writes
                    nc.vector.scalar_tensor_tensor(
                        out=acc[:], in0=mem[0][:], scalar=TAU,
                        in1=xs, op0=mult, op1=add,
                    )
                else:
                    for h in range(B_SH):
                        hs = slice(h * FB, (h + 1) * FB)
                        # acc = (mem * tau) + x_t  (independent per half)
                        nc.vector.scalar_tensor_tensor(
                            out=acc[:, hs],
                            in0=mem[h][:, hs] if (ACC_FULL or MEM_FULL)
                            else mem[h][:], scalar=TAU,
                            in1=xs[:, hs], op0=mult, op1=add,
                        )
            afull = acc if t == 0 else acc[:]
            if t < T - 1:
                if ACC_FULL:
                    m = mps[0].tile([P, F], f32, name="memf")
                    for h in range(B_SH):
                        hs = slice(h * FB, (h + 1) * FB)
                        # mem' = (acc <= vth) * acc   (hard reset)
                        nc.vector.scalar_tensor_tensor(
                            out=m[:, hs], in0=afull[:, hs], scalar=VTH,
                            in1=afull[:, hs], op0=is_le, op1=mult,
                        )
                    mem = [m, m]
                elif MEM_FULL:
                    # one full-tile mem op; acc halves read their slices
                    m = mps[0].tile([P, F], f32, name="memf")
                    nc.vector.scalar_tensor_tensor(
                        out=m[:], in0=afull, scalar=VTH,
                        in1=afull, op0=is_le, op1=mult,
                    )
                    mem = [m, m]
                else:
                    for h in range(B_SH):
                        hs = slice(h * FB, (h + 1) * FB)
                        m = mps[h].tile([P, FB], f32)
                        # mem' = (acc <= vth) * acc   (hard reset)
                        nc.vector.scalar_tensor_tensor(
                            out=m[:], in0=afull[:, hs], scalar=VTH,
                            in1=afull[:, hs], op0=is_le, op1=mult,
                        )
                        mem[h] = m
            # spike = sign(acc-vth) written directly as uint8: the f32->u8
            # conversion saturates -1 to 0, so u8(sgn) is exactly the
            # {0,1} spike (verified incl. threshold-boundary values)
            nc.scalar.activation(spk[:, tl * F:(tl + 1) * F], afull,
                                 Sign, bias=nvth[:])
        if mode == "full":
            st_eng = {"sync": nc.sync, "scalar": nc.scalar,
                      "pool": nc.gpsimd}[ST_RING]
            st_eng.dma_start(out=out[g], in_=spk[:])


def _get_program(hw_loop=None, mode="full"):
    key = (hw_loop, mode)
    if key not in _progs:
        _progs[key] = _build_program(hw_loop, mode)
    return _progs[key]


# ---- host-side shard/gather ------------------------------------------

def _shard_input(xc):
    return np.ascontiguousarray(xc)


def _gather_output(oc):
    """[NG, P, JG] spike-dtype -> [B_SH, T, D] f32 spikes (exact)."""
    oc = np.asarray(oc)
    if oc.dtype == np.uint8 and SPK_DT == "f8":
        oc = (oc != 0)
    elif SPK_DT == "f8":
        oc = (np.asarray(oc).view(np.uint8) != 0)
    sp = oc.reshape(NG, P, GS, B_SH, FB).transpose(3, 0, 2, 1, 4)
    return np.ascontiguousarray(
        sp.reshape(B_SH, T, D).astype(np.float32)
    )


def device_input(x):
    """Full [B, T, D] -> axis-0 shard-concatenated device input array."""
    return np.ascontiguousarray(np.asarray(x, dtype=np.float32))


def device_output(o):
    """Axis-0 shard-concatenated device output -> full [B, T, D] f32."""
    rows = o.shape[0] // N_CORES
    return np.concatenate(
        [
            _gather_output(o[i * rows:(i + 1) * rows])
            for i in range(N_CORES)
        ],
        axis=0,
    )


def _shard(x):
    return [
        {"x": _shard_input(x[i * B_SH:(i + 1) * B_SH])}
        for i in range(N_CORES)
    ]


def kernel(x):
    x = np.asarray(x, dtype=np.float32)
    assert x.shape == (B, T, D), x.shape
    nc = _get_program()
    res = run_bass_kernel_spmd(nc, _shard(x), list(range(N_CORES)))
    return np.concatenate(
        [_gather_output(res.results[i]["out"]) for i in range(N_CORES)],
        axis=0,
    )


# revision 21
# speedup vs baseline: 1.0052x; 1.0052x over previous
"""LIF neuron (leaky integrate-and-fire) Bass kernel for Trainium2.

Reference semantics (per element, recurrence over time axis T=32):
    mem_t   = tau * mem_{t-1} + x_t
    spike_t = 1.0 if mem_t > vth else 0.0
    mem_t   = mem_t * (1 - spike_t)        # hard reset
Input  x: [16, 32, 65536] f32  ->  Output spikes: [16, 32, 65536] f32.
Sharding: data parallel over batch, 8 cores x 2 batch rows each.

Design (v3, measured-engine-balanced):
  The recurrence needs two 2-tensor DVE ops per step (acc, mem) --
  ~77us/pass on DVE at 1 elem/lane/cycle; that's the compute floor and
  the binding engine.  Spike extraction rides ACT: ONE Sign op per
  step writing uint8 directly -- the f32->u8 conversion saturates the
  sign's -1 to 0, so u8(sign(acc-vth)) IS the {0,1} spike exactly
  (~34us ACT).  Stores shrink 4x (16 MiB -> 4 MiB/core) and ride the
  scalar HWDGE ring, overlapping the sync-ring load stream.
  Pool/PE idle: Pool's Q7 software TT is ~4x slower than DVE and
  walrus rejects STT/comparisons on it; PE fp32 matmul is ~1/4 rate.

  Membrane chain is split into two independent batch-row halves
  [128, 512] so the serial acc->mem->acc dependency pipelines on DVE
  (chain ~46us < 77us throughput).

Per step, on a [128, 1024] step-tile (2 batch x 512 d per partition):
  DVE  STT x2: acc_h = (mem_h * tau) + x_h          (per half, f32)
  DVE  STT x2: mem_h' = (acc_h <= vth) * acc_h      (per half, f32)
  ACT  Sign:   spk_u8 = u8(sign(acc - vth))         (uint8 out, exact)
Ring schedule per pass: all 16x1MiB loads issue first on the sync
HWDGE ring; per-group 512KiB uint8 spike stores ride the scalar ring
concurrently.  Full-x SBUF residency (bufs=NG).
"""

import os
import sys

sys.path.insert(0, "/opt/trn_rl_repo")

import numpy as np

from concourse import bacc, mybir, tile
from concourse.bass_utils import run_bass_kernel_spmd

TAU = 0.2
VTH = 0.5

B, T, D = 16, 32, 65536
N_CORES = 8
B_SH = B // N_CORES          # 2 batch rows per core
P = 128                      # SBUF partitions
FB = D // P                  # 512 d-elems per partition per batch row
F = B_SH * FB                # 1024 free elems per step-tile

GS = int(os.environ.get("LIF_GS", "4"))   # timesteps per DMA group
ACC_BUFS = int(os.environ.get("LIF_ACCBUFS", "4"))
MEM_BUFS = int(os.environ.get("LIF_MEMBUFS", "4"))
SPK_BUFS = int(os.environ.get("LIF_SPKBUFS", "3"))
NG = T // GS                 # groups per pass
JG = GS * F                  # per-group free elems (4096)

SPK_DT = os.environ.get("LIF_SPKDT", "u8")   # u8 | f8 | bf16
SPLIT0 = int(os.environ.get("LIF_SPLIT0", "0"))  # per-step DMAs for group 0
ST_RING = os.environ.get("LIF_STRING", "scalar")  # sync | scalar | pool
ACC_FULL = int(os.environ.get("LIF_ACCFULL", "0"))  # acc as one [P,F] op
MEM_FULL = int(os.environ.get("LIF_MEMFULL", "0"))  # mem as one [P,F] op
SPK_HALF = int(os.environ.get("LIF_SPKHALF", "0"))  # spike Sign per half

_progs = {}


def _spk_dt():
    return {"u8": mybir.dt.uint8, "f8": mybir.dt.float8e4,
            "bf16": mybir.dt.bfloat16}[SPK_DT]


def _build_program(hw_loop=None, mode="full"):
    f32 = mybir.dt.float32
    nc = bacc.Bacc(
        "TRN2",
        target_bir_lowering=False,
        debug=False,
        enable_asserts=False,
        num_devices=N_CORES,
    )
    x = nc.dram_tensor("x", [B_SH, T, D], f32, kind="ExternalInput").ap()
    xr = x.rearrange("b (g tl) (p f) -> g p tl b f", tl=GS, p=P)
    out = nc.dram_tensor("out", [NG, P, JG], _spk_dt(),
                         kind="ExternalOutput").ap()

    with tile.TileContext(nc) as tc:
        with (
            # full-x residency: one buffer per group, reused across passes
            tc.tile_pool(name="xt", bufs=NG) as xp,
            tc.tile_pool(name="acc", bufs=ACC_BUFS) as ap_,
            tc.tile_pool(name="spk", bufs=SPK_BUFS) as kp,
            tc.tile_pool(name="m0", bufs=MEM_BUFS) as mp0,
            tc.tile_pool(name="m1", bufs=MEM_BUFS) as mp1,
            tc.tile_pool(name="const", bufs=1) as cp,
        ):
            nvth = cp.tile([P, 1], f32)
            nc.gpsimd.memset(nvth[:], -VTH)

            def body():
                one_pass(nc, xr, out, xp, ap_, kp, (mp0, mp1),
                         nvth, mode)

            if hw_loop is None:
                body()
            else:
                # benchmarking only: repeat the full pass in a HW loop so
                # per-pass device time can be fit from wall-clock deltas
                with tc.For_i(0, hw_loop, 1):
                    body()
    nc.compile()
    return nc


def one_pass(nc, xr, out, xp, ap_, kp, mps, nvth, mode):
    f32 = mybir.dt.float32
    mult = mybir.AluOpType.mult
    add = mybir.AluOpType.add
    is_le = mybir.AluOpType.is_le
    Sign = mybir.ActivationFunctionType.Sign

    # Phase 1: issue every load on the sync ring (back-to-back, no WAR).
    # Group 0 loads are split per-step so the first acc can start after
    # a 256 KiB DMA instead of a 1 MiB one (ramp cut).
    xts = []
    for g in range(NG):
        xt = xp.tile([P, JG], f32)
        if mode == "compute":
            nc.gpsimd.memset(xt[:], 0.125)
        else:
            xt_v = xt[:].rearrange("p (tl b f) -> p tl b f", tl=GS, b=B_SH)
            if g == 0 and SPLIT0:
                for tl in range(GS):
                    for b in range(B_SH):
                        nc.sync.dma_start(out=xt_v[:, tl, b],
                                          in_=xr[g][:, tl, b])
            else:
                for b in range(B_SH):
                    nc.sync.dma_start(out=xt_v[:, :, b], in_=xr[g][:, :, b])
        xts.append(xt)
    if mode == "load":
        return

    # Phase 2: recurrence; uint8 spike stores drain after loads by FIFO
    mem = [None, None]           # per-half membrane tiles [P, FB]
    for g in range(NG):
        xt = xts[g]
        spk = kp.tile([P, JG], _spk_dt())
        for tl in range(GS):
            t = g * GS + tl
            xs = xt[:, tl * F:(tl + 1) * F]
            if t == 0:
                acc = xs         # mem_{-1} = 0 -> acc = x_0
            else:
                acc = ap_.tile([P, F], f32)
                if ACC_FULL:
                    # one full-tile op; mem is a full tile w/ half writes
                    nc.vector.scalar_tensor_tensor(
                        out=acc[:], in0=mem[0][:], scalar=TAU,
                        in1=xs, op0=mult, op1=add,
                    )
                else:
                    for h in range(B_SH):
                        hs = slice(h * FB, (h + 1) * FB)
                        # acc = (mem * tau) + x_t  (independent per half)
                        nc.vector.scalar_tensor_tensor(
                            out=acc[:, hs],
                            in0=mem[h][:, hs] if (ACC_FULL or MEM_FULL)
                            else mem[h][:], scalar=TAU,
                            in1=xs[:, hs], op0=mult, op1=add,
                        )
            afull = acc if t == 0 else acc[:]
            if t < T - 1:
                if ACC_FULL:
                    m = mps[0].tile([P, F], f32, name="memf")
                    for h in range(B_SH):
                        hs = slice(h * FB, (h + 1) * FB)
                        # mem' = (acc <= vth) * acc   (hard reset)
                        nc.vector.scalar_tensor_tensor(
                            out=m[:, hs], in0=afull[:, hs], scalar=VTH,
                            in1=afull[:, hs], op0=is_le, op1=mult,
                        )
                    mem = [m, m]
                elif MEM_FULL:
                    # one full-tile mem op; acc halves read their slices
                    m = mps[0].tile([P, F], f32, name="memf")
                    nc.vector.scalar_tensor_tensor(
                        out=m[:], in0=afull, scalar=VTH,
                        in1=afull, op0=is_le, op1=mult,
                    )
                    mem = [m, m]
                else:
                    for h in range(B_SH):
                        hs = slice(h * FB, (h + 1) * FB)
                        m = mps[h].tile([P, FB], f32)
                        # mem' = (acc <= vth) * acc   (hard reset)
                        nc.vector.scalar_tensor_tensor(
                            out=m[:], in0=afull[:, hs], scalar=VTH,
                            in1=afull[:, hs], op0=is_le, op1=mult,
                        )
                        mem[h] = m
            # spike = sign(acc-vth) written directly as uint8: the f32->u8
            # conversion saturates -1 to 0, so u8(sgn) is exactly the
            # {0,1} spike (verified incl. threshold-boundary values)
            if SPK_HALF:
                for h in range(B_SH):
                    hs = slice(h * FB, (h + 1) * FB)
                    nc.scalar.activation(
                        spk[:, tl * F + h * FB: tl * F + (h + 1) * FB],
                        afull[:, hs], Sign, bias=nvth[:])
            else:
                nc.scalar.activation(spk[:, tl * F:(tl + 1) * F], afull,
                                     Sign, bias=nvth[:])
        if mode == "full":
            st_eng = {"sync": nc.sync, "scalar": nc.scalar,
                      "pool": nc.gpsimd}[ST_RING]
            st_eng.dma_start(out=out[g], in_=spk[:])


def _get_program(hw_loop=None, mode="full"):
    key = (hw_loop, mode)
    if key not in _progs:
        _progs[key] = _build_program(hw_loop, mode)
    return _progs[key]


# ---- host-side shard/gather ------------------------------------------

def _shard_input(xc):
    return np.ascontiguousarray(xc)


def _gather_output(oc):
    """[NG, P, JG] spike-dtype -> [B_SH, T, D] f32 spikes (exact)."""
    oc = np.asarray(oc)
    if oc.dtype == np.uint8 and SPK_DT == "f8":
        oc = (oc != 0)
    elif SPK_DT == "f8":
        oc = (np.asarray(oc).view(np.uint8) != 0)
    sp = oc.reshape(NG, P, GS, B_SH, FB).transpose(3, 0, 2, 1, 4)
    return np.ascontiguousarray(
        sp.reshape(B_SH, T, D).astype(np.float32)
    )


def device_input(x):
    """Full [B, T, D] -> axis-0 shard-concatenated device input array."""
    return np.ascontiguousarray(np.asarray(x, dtype=np.float32))


def device_output(o):
    """Axis-0 shard-concatenated device output -> full [B, T, D] f32."""
    rows = o.shape[0] // N_CORES
    return np.concatenate(
        [
            _gather_output(o[i * rows:(i + 1) * rows])
            for i in range(N_CORES)
        ],
        axis=0,
    )


def _shard(x):
    return [
        {"x": _shard_input(x[i * B_SH:(i + 1) * B_SH])}
        for i in range(N_CORES)
    ]


def kernel(x):
    x = np.asarray(x, dtype=np.float32)
    assert x.shape == (B, T, D), x.shape
    nc = _get_program()
    res = run_bass_kernel_spmd(nc, _shard(x), list(range(N_CORES)))
    return np.concatenate(
        [_gather_output(res.results[i]["out"]) for i in range(N_CORES)],
        axis=0,
    )
